# revision 4
# baseline (speedup 1.0000x reference)
"""Trainium2 Bass kernel for nn_ETypePromptModel: logits = einsum('bpd,cpd->bc').

Equivalent to X @ W.T with X=[B, K]=[16384, 256], W=[C, K]=[4096, 256],
K = L*D = 256. Data-parallel over B across 8 NeuronCores; W replicated.

bf16 plan (rel-err gate is 2e-2; bf16 end-to-end lands ~3e-3):
  - Host casts X/W to bf16 and lays them out K-major ([K, B_loc] / [K, C])
    so the kernel needs no on-device transposes; output is written bf16
    and upcast to fp32 on the host. Per-core DRAM traffic drops from
    39.5 MB (fp32) to 19.8 MB -- input 3 MB + output 16.8 MB.
  - PE: 256 bf16 matmuls ([128k x 128b] stationary, [128k x 512c]
    moving, fp32 PSUM, k0/k1 back-to-back per bank) at the 216 ns/MM
    streaming rate = 55.3 us warm @ 2.4 GHz -- the bf16 ridge
    (78.6 TF/s / 358 GB/s ~ 219 flop/B vs 217 here), so PE and HBM
    are simultaneously near-saturated.
  - The c-dimension is processed in two half-column passes (jh-outer):
    the stream's startup-critical input set is only W-half of both
    k-tiles + the first X columns (~1.25 MB), cutting the wait on the
    ~310 GB/s input-DMA phase vs needing all of W up front.
  - X is loaded via the GpSimd SWDGE ring (its queue starts ~1 us
    earlier and adds a third DMA queue); W halves go one per HWDGE
    ring, late-needed halves deferred behind them.
  - Junk warmup matmuls keep the PE busy from the preamble end until
    the stream starts, so the HAM clock gate is 8/8 from the first
    real matmul.
  - PSUM -> SBUF copies (cast to bf16) alternate Vector/Scalar and are
    spread evenly by the per-bank k-interleaving; output goes out as
    0.5 MB half-row DMAs alternating the two HWDGE rings.
"""

import sys

import numpy as np

sys.path.insert(0, "/opt/trn_rl_repo")

B, C, L, D = 16384, 4096, 2, 128
K = L * D  # 256 contraction length
N_CORES = 8
B_LOC = B // N_CORES  # 2048
P = 128
KT = K // P  # 2 k-tiles
M_TILES = B_LOC // P  # 16
N_TILE = 512  # moving free dim per matmul (PSUM bank = 512 fp32)
JH_TILES = 4  # c-tiles per half-column pass
WARMUP_MMS = 14

_CACHE = {}
PROFILE = False
TRACE_ALL_CORES = False
LAST_RESULT = None


def _build():
    import concourse.mybir as mybir
    import concourse.tile as tile
    from concourse import bacc

    f32 = mybir.dt.float32
    bf16 = mybir.dt.bfloat16

    nc = bacc.Bacc(
        "TRN2",
        target_bir_lowering=False,
        debug=False,
        enable_asserts=False,
        num_devices=N_CORES,
    )

    xt_dram = nc.dram_tensor("xt", [K, B_LOC], bf16, kind="ExternalInput").ap()
    wt_dram = nc.dram_tensor("wt", [K, C], bf16, kind="ExternalInput").ap()
    out_dram = nc.dram_tensor("out", [B_LOC, C], bf16, kind="ExternalOutput").ap()

    CH = C // 2  # 2048
    XP = 512  # first-X chunk (stationaries for m-tiles 0-3)

    with tile.TileContext(nc) as tc:
        with (
            tc.tile_pool(name="cst", bufs=1) as cst_pool,
            tc.tile_pool(name="big", bufs=1) as big_pool,
            tc.tile_pool(name="osb", bufs=6) as out_pool,
            tc.tile_pool(name="psm", bufs=8, space="PSUM") as psum_pool,
        ):
            # --- PE warmup: junk matmuls keep HAM at 8/8 until inputs land
            junk = cst_pool.tile([P, N_TILE], bf16, name="junk")
            nc.vector.memset(junk, 0.0)
            warm_ps = psum_pool.tile([P, N_TILE], f32, tag="pmm", name="warm_ps")
            for _ in range(WARMUP_MMS):
                nc.tensor.matmul(warm_ps, junk[:, :P], junk, start=True, stop=True)

            # --- input loads
            xt_sb = [
                big_pool.tile([P, B_LOC], bf16, name=f"xt{k}") for k in range(KT)
            ]
            wt_sb = [big_pool.tile([P, C], bf16, name=f"wt{k}") for k in range(KT)]

            # X on the SWDGE ring: first columns first, rest behind
            for k in range(KT):
                nc.gpsimd.dma_start(
                    xt_sb[k][:, 0:XP], xt_dram[k * P : (k + 1) * P, 0:XP]
                )
            for k in range(KT):
                nc.gpsimd.dma_start(
                    xt_sb[k][:, XP:B_LOC], xt_dram[k * P : (k + 1) * P, XP:B_LOC]
                )
            # W: one k-tile per HWDGE ring; jh1 halves deferred behind jh0
            w_rings = (nc.sync, nc.scalar)
            for jh in range(2):
                for k in range(KT):
                    w_rings[k].dma_start(
                        wt_sb[k][:, jh * CH : (jh + 1) * CH],
                        wt_dram[k * P : (k + 1) * P, jh * CH : (jh + 1) * CH],
                    )

            # --- main stream: two half-column passes over the m-tiles
            for jh in range(2):
                for mt in range(M_TILES):
                    pms = [
                        psum_pool.tile([P, N_TILE], f32, tag="pmm", name="pmm")
                        for _ in range(JH_TILES)
                    ]
                    stats = [xt_sb[k][:, mt * P : (mt + 1) * P] for k in range(KT)]
                    for jj in range(JH_TILES):
                        j = jh * JH_TILES + jj
                        for k in range(KT):
                            nc.tensor.matmul(
                                pms[jj],
                                stats[k],
                                wt_sb[k][:, j * N_TILE : (j + 1) * N_TILE],
                                start=(k == 0),
                                stop=(k == KT - 1),
                            )

                    out_sb = out_pool.tile([P, CH], bf16, tag="osb", name="out_sb")
                    row = out_dram[mt * P : (mt + 1) * P, jh * CH : (jh + 1) * CH]
                    last = jh == 1 and mt == M_TILES - 1
                    for jj in range(JH_TILES):
                        sl = slice(jj * N_TILE, (jj + 1) * N_TILE)
                        if jj % 2 == 0:
                            nc.vector.tensor_copy(out=out_sb[:, sl], in_=pms[jj])
                        else:
                            nc.scalar.copy(out_sb[:, sl], pms[jj])
                        if last and jj == 1:
                            # split the final write so its DMA overlaps the
                            # last two copies
                            nc.sync.dma_start(
                                row[:, 0 : 2 * N_TILE], out_sb[:, 0 : 2 * N_TILE]
                            )
                    if last:
                        nc.scalar.dma_start(
                            row[:, 2 * N_TILE : CH], out_sb[:, 2 * N_TILE : CH]
                        )
                    else:
                        ring = nc.sync if mt % 2 == 0 else nc.scalar
                        ring.dma_start(row, out_sb)

    nc.compile()
    return nc


def kernel(batchs, label2embed):
    global LAST_RESULT
    import ml_dtypes

    from concourse.bass_utils import run_bass_kernel_spmd

    bf16 = ml_dtypes.bfloat16

    if "nc" not in _CACHE:
        _CACHE["nc"] = _build()
    nc = _CACHE["nc"]

    X = np.ascontiguousarray(batchs, dtype=np.float32).reshape(B, K)
    W = np.ascontiguousarray(label2embed, dtype=np.float32).reshape(C, K)
    assert X.shape == (B, K) and W.shape == (C, K)

    wt = np.ascontiguousarray(W.astype(bf16).T)  # [K, C]
    Xb = X.astype(bf16)
    in_maps = [
        {
            "xt": np.ascontiguousarray(Xb[c * B_LOC : (c + 1) * B_LOC].T),
            "wt": wt,
        }
        for c in range(N_CORES)
    ]
    res = run_bass_kernel_spmd(
        nc,
        in_maps,
        core_ids=list(range(N_CORES)),
        trace=PROFILE,
        trace_cores=list(range(N_CORES)) if (PROFILE and TRACE_ALL_CORES) else None,
    )
    LAST_RESULT = res
    out = np.concatenate([r["out"] for r in res.results], axis=0)
    return out.astype(np.float32)


# revision 5
# speedup vs baseline: 1.1056x; 1.1056x over previous
"""Trainium2 Bass kernel for nn_ETypePromptModel: logits = einsum('bpd,cpd->bc').

Equivalent to X @ W.T with X=[B, K]=[16384, 256], W=[C, K]=[4096, 256],
K = L*D = 256. Data-parallel over B across 8 NeuronCores; W replicated.

bf16 plan (rel-err gate is 2e-2; bf16 end-to-end lands ~3e-3):
  - Host casts X/W to bf16 and lays them out K-major ([K, B_loc] / [K, C])
    so the kernel needs no on-device transposes; output is written bf16
    and upcast to fp32 on the host. Per-core DRAM traffic drops from
    39.5 MB (fp32) to 19.8 MB -- input 3 MB + output 16.8 MB.
  - PE: 256 bf16 matmuls ([128k x 128b] stationary, [128k x 512c]
    moving, fp32 PSUM, k0/k1 back-to-back per bank) at the 216 ns/MM
    streaming rate = 55.3 us warm @ 2.4 GHz -- the bf16 ridge
    (78.6 TF/s / 358 GB/s ~ 219 flop/B vs 217 here), so PE and HBM
    are simultaneously near-saturated.
  - The c-dimension is processed in two half-column passes (jh-outer):
    the stream's startup-critical input set is only W-half of both
    k-tiles + the first X columns (~1.25 MB), cutting the wait on the
    ~310 GB/s input-DMA phase vs needing all of W up front.
  - X is loaded via the GpSimd SWDGE ring (its queue starts ~1 us
    earlier and adds a third DMA queue); W halves go one per HWDGE
    ring, late-needed halves deferred behind them.
  - Junk warmup matmuls keep the PE busy from the preamble end until
    the stream starts, so the HAM clock gate is 8/8 from the first
    real matmul.
  - PSUM -> SBUF copies (cast to bf16) alternate Vector/Scalar and are
    spread evenly by the per-bank k-interleaving; output goes out as
    0.5 MB half-row DMAs alternating the two HWDGE rings.
"""

import sys

import numpy as np

sys.path.insert(0, "/opt/trn_rl_repo")

B, C, L, D = 16384, 4096, 2, 128
K = L * D  # 256 contraction length
N_CORES = 8
B_LOC = B // N_CORES  # 2048
P = 128
KT = K // P  # 2 k-tiles
M_TILES = B_LOC // P  # 16
N_TILE = 512  # moving free dim per matmul (PSUM bank = 512 fp32)
JH_TILES = 4  # c-tiles per half-column pass
WARMUP_MMS = 15

_CACHE = {}
PROFILE = False
TRACE_ALL_CORES = False
LAST_RESULT = None


def _build():
    import concourse.mybir as mybir
    import concourse.tile as tile
    from concourse import bacc

    f32 = mybir.dt.float32
    bf16 = mybir.dt.bfloat16

    nc = bacc.Bacc(
        "TRN2",
        target_bir_lowering=False,
        debug=False,
        enable_asserts=False,
        num_devices=N_CORES,
    )

    xt_dram = nc.dram_tensor("xt", [K, B_LOC], bf16, kind="ExternalInput").ap()
    wt_dram = nc.dram_tensor("wt", [K, C], bf16, kind="ExternalInput").ap()
    out_dram = nc.dram_tensor("out", [B_LOC, C], bf16, kind="ExternalOutput").ap()

    CH = C // 2  # 2048
    XP = 512  # first-X chunk (stationaries for m-tiles 0-3)

    with tile.TileContext(nc) as tc:
        with (
            tc.tile_pool(name="cst", bufs=1) as cst_pool,
            tc.tile_pool(name="big", bufs=1) as big_pool,
            tc.tile_pool(name="osb", bufs=6) as out_pool,
            tc.tile_pool(name="psm", bufs=8, space="PSUM") as psum_pool,
        ):
            # --- PE warmup: junk matmuls keep HAM at 8/8 until inputs land
            junk = cst_pool.tile([P, N_TILE], bf16, name="junk")
            nc.vector.memset(junk, 0.0)
            warm_ps = psum_pool.tile([P, N_TILE], f32, tag="pmm", name="warm_ps")
            for _ in range(WARMUP_MMS):
                nc.tensor.matmul(warm_ps, junk[:, :P], junk, start=True, stop=True)

            # --- input loads
            xt_sb = [
                big_pool.tile([P, B_LOC], bf16, name=f"xt{k}") for k in range(KT)
            ]
            wt_sb = [big_pool.tile([P, C], bf16, name=f"wt{k}") for k in range(KT)]

            # X first columns on the SWDGE ring (starts early, tiny);
            # each HWDGE ring then carries its k-tile: W h0 (startup
            # critical), X rest (needed from m-tile 4), W h1 (second pass)
            for k in range(KT):
                nc.gpsimd.dma_start(
                    xt_sb[k][:, 0:XP], xt_dram[k * P : (k + 1) * P, 0:XP]
                )
            w_rings = (nc.sync, nc.scalar)
            for k in range(KT):
                w_rings[k].dma_start(
                    wt_sb[k][:, 0:CH], wt_dram[k * P : (k + 1) * P, 0:CH]
                )
            for k in range(KT):
                w_rings[k].dma_start(
                    xt_sb[k][:, XP:B_LOC], xt_dram[k * P : (k + 1) * P, XP:B_LOC]
                )
            for k in range(KT):
                w_rings[k].dma_start(
                    wt_sb[k][:, CH:C], wt_dram[k * P : (k + 1) * P, CH:C]
                )

            # --- main stream: two half-column passes over the m-tiles
            for jh in range(2):
                for mt in range(M_TILES):
                    pms = [
                        psum_pool.tile([P, N_TILE], f32, tag="pmm", name="pmm")
                        for _ in range(JH_TILES)
                    ]
                    stats = [xt_sb[k][:, mt * P : (mt + 1) * P] for k in range(KT)]
                    for jj in range(JH_TILES):
                        j = jh * JH_TILES + jj
                        for k in range(KT):
                            nc.tensor.matmul(
                                pms[jj],
                                stats[k],
                                wt_sb[k][:, j * N_TILE : (j + 1) * N_TILE],
                                start=(k == 0),
                                stop=(k == KT - 1),
                            )

                    out_sb = out_pool.tile([P, CH], bf16, tag="osb", name="out_sb")
                    row = out_dram[mt * P : (mt + 1) * P, jh * CH : (jh + 1) * CH]
                    last = jh == 1 and mt == M_TILES - 1
                    for jj in range(JH_TILES):
                        sl = slice(jj * N_TILE, (jj + 1) * N_TILE)
                        if jj % 2 == 0:
                            nc.vector.tensor_copy(out=out_sb[:, sl], in_=pms[jj])
                        else:
                            nc.scalar.copy(out_sb[:, sl], pms[jj])
                        if last and jj == 1:
                            # split the final write so its DMA overlaps the
                            # last two copies
                            nc.sync.dma_start(
                                row[:, 0 : 2 * N_TILE], out_sb[:, 0 : 2 * N_TILE]
                            )
                    if last:
                        nc.gpsimd.dma_start(
                            row[:, 2 * N_TILE : CH], out_sb[:, 2 * N_TILE : CH]
                        )
                    else:
                        # scalar issues no DMAs: a dma_start waiting on the
                        # vector-side copies would head-of-line block the
                        # next m-tile's scalar copies
                        ring = nc.sync if mt % 2 == 0 else nc.gpsimd
                        ring.dma_start(row, out_sb)

    nc.compile()
    return nc


def kernel(batchs, label2embed):
    global LAST_RESULT
    import ml_dtypes

    from concourse.bass_utils import run_bass_kernel_spmd

    bf16 = ml_dtypes.bfloat16

    if "nc" not in _CACHE:
        _CACHE["nc"] = _build()
    nc = _CACHE["nc"]

    X = np.ascontiguousarray(batchs, dtype=np.float32).reshape(B, K)
    W = np.ascontiguousarray(label2embed, dtype=np.float32).reshape(C, K)
    assert X.shape == (B, K) and W.shape == (C, K)

    wt = np.ascontiguousarray(W.astype(bf16).T)  # [K, C]
    Xb = X.astype(bf16)
    in_maps = [
        {
            "xt": np.ascontiguousarray(Xb[c * B_LOC : (c + 1) * B_LOC].T),
            "wt": wt,
        }
        for c in range(N_CORES)
    ]
    res = run_bass_kernel_spmd(
        nc,
        in_maps,
        core_ids=list(range(N_CORES)),
        trace=PROFILE,
        trace_cores=list(range(N_CORES)) if (PROFILE and TRACE_ALL_CORES) else None,
    )
    LAST_RESULT = res
    out = np.concatenate([r["out"] for r in res.results], axis=0)
    return out.astype(np.float32)
